# revision 11
# baseline (speedup 1.0000x reference)
"""GroupNorm + 4-head self-attention + output projection, TRN2 Bass kernel.

Sharding: 8 cores = 4 batches x 2 query-halves.  Each core runs GroupNorm and
the full K/V projection for its batch (duplicated across the 2 cores of a
batch, ~5% extra FLOPs) and attention + output projection for its 2048-query
chunk.  The query chunk is rotated to the front of the token axis on the host
(GroupNorm stats / K / V are permutation-invariant along tokens), so all 8
cores run one identical SPMD program and the unshard is pure concatenation.

Device layout (per core, all fp32):
  x        [256, 4096]  channels on partitions (2 tiles of 128)
  GroupNorm: per-channel bn_stats -> group merge via tiny block-ones matmul
             -> rstd -> broadcast back via tiny matmul -> fused scale/bias
  q^T,k^T  [128, nq/4096] via w_qkv^T-stationary matmuls
  v        [m,128] via x-stationary matmuls (row layout needed for attn@V)
  sim^T    [m-chunk=128, 512] per head, K=32 matmuls packed 4-up with
           tile_position row tiling; exp on ACT (scale=1/sqrt(32) folded in)
  attn@V   col-tiled 4-up (M=32/head) accumulated over m in PSUM
  denom    ones-vector col-tiled matmuls accumulated in PSUM
  out      attnout = oacc * bcast(1/denom); y = w_out^T.T @ attnout + b_out
"""

import numpy as np

HEAD = 4
DIM_HEAD = 32
DIM = 256
GROUPS = 32
EPS = 1e-5
SCALE = DIM_HEAD ** -0.5
N = 4096
NQ = 2048
NCORES = 8
P = 128
JW = 512           # query-chunk width per inner tile
NJ = NQ // JW      # 4
NI = N // P        # 32 key chunks

_cache = {}


def _get_nc():
    if "nc" in _cache:
        return _cache["nc"]
    from contextlib import ExitStack

    import concourse.bass as bass  # noqa: F401
    import concourse.tile as tile
    from concourse import bacc, mybir

    f32 = mybir.dt.float32
    AF = mybir.ActivationFunctionType
    ALU = mybir.AluOpType

    nc = bacc.Bacc(None, target_bir_lowering=False)
    x_in = nc.declare_dram_parameter("x", [DIM, N], f32, isOutput=False)
    wqkvT = nc.declare_dram_parameter("wqkvT", [DIM, 3 * P], f32, isOutput=False)
    woutT = nc.declare_dram_parameter("woutT", [P, DIM], f32, isOutput=False)
    gnw = nc.declare_dram_parameter("gnw", [DIM, 1], f32, isOutput=False)
    gnb = nc.declare_dram_parameter("gnb", [DIM, 1], f32, isOutput=False)
    bout = nc.declare_dram_parameter("bout", [DIM, 1], f32, isOutput=False)
    blk8 = nc.declare_dram_parameter("blk8", [P, 16], f32, isOutput=False)
    blk8T = nc.declare_dram_parameter("blk8T", [16, P], f32, isOutput=False)
    e4 = nc.declare_dram_parameter("e4", [P, P], f32, isOutput=False)
    onesP = nc.declare_dram_parameter("onesP", [P, 1], f32, isOutput=False)
    y_out = nc.declare_dram_parameter("y", [DIM, NQ], f32, isOutput=True)

    with ExitStack() as ctx:
        tc = ctx.enter_context(tile.TileContext(nc))
        const = ctx.enter_context(tc.tile_pool(name="const", bufs=1))
        persist = ctx.enter_context(tc.tile_pool(name="persist", bufs=1))
        work = ctx.enter_context(tc.tile_pool(name="work", bufs=3))
        attnp = ctx.enter_context(tc.tile_pool(name="attnp", bufs=3))
        # PSUM budget (8 banks): sim 2 slots x 2 banks + oacc 2 x 1 + dn 2 x 1
        psA = ctx.enter_context(tc.tile_pool(name="psA", bufs=2, space="PSUM"))
        psB = ctx.enter_context(tc.tile_pool(name="psB", bufs=2, space="PSUM"))

        # ---------------- constants ----------------
        wqkv_sb = []
        for t in range(2):
            w = const.tile([P, 3 * P], f32, tag=f"wqkv{t}")
            nc.sync.dma_start(out=w, in_=wqkvT[t * P:(t + 1) * P, :])
            wqkv_sb.append(w)
        wout_sb = const.tile([P, DIM], f32, tag="wout")
        nc.sync.dma_start(out=wout_sb, in_=woutT[:, :])
        gnw_sb, gnb_sb, bout_sb = [], [], []
        for t in range(2):
            a = const.tile([P, 1], f32, tag=f"gnw{t}")
            nc.sync.dma_start(out=a, in_=gnw[t * P:(t + 1) * P, :])
            gnw_sb.append(a)
            b = const.tile([P, 1], f32, tag=f"gnb{t}")
            nc.sync.dma_start(out=b, in_=gnb[t * P:(t + 1) * P, :])
            gnb_sb.append(b)
            c0 = const.tile([P, 1], f32, tag=f"bout{t}")
            nc.sync.dma_start(out=c0, in_=bout[t * P:(t + 1) * P, :])
            bout_sb.append(c0)
        blk8_sb = const.tile([P, 16], f32, tag="blk8")
        nc.sync.dma_start(out=blk8_sb, in_=blk8[:, :])
        blk8T_sb = const.tile([16, P], f32, tag="blk8T")
        nc.sync.dma_start(out=blk8T_sb, in_=blk8T[:, :])
        e4_sb = const.tile([P, P], f32, tag="e4")
        nc.sync.dma_start(out=e4_sb, in_=e4[:, :])
        dnc = const.tile([P, JW], f32, tag="dnc")
        nc.vector.memset(dnc, 1.0)
        ones_sb = const.tile([P, 1], f32, tag="ones")
        nc.sync.dma_start(out=ones_sb, in_=onesP[:, :])
        eps_sb = const.tile([16, 1], f32, tag="eps")
        nc.vector.memset(eps_sb, EPS)

        # ---------------- load x (chunked so bn_stats can overlap) ----------
        xc = []
        for t in range(2):
            xt = persist.tile([P, N], f32, tag=f"xc{t}")
            for ch in range(8):
                nc.sync.dma_start(
                    out=xt[:, ch * 512:(ch + 1) * 512],
                    in_=x_in[t * P:(t + 1) * P, ch * 512:(ch + 1) * 512],
                )
            xc.append(xt)

        # ---------------- GroupNorm ----------------
        for t in range(2):
            stats = work.tile([P, 8, 6], f32, tag="stats")
            for ch in range(8):
                nc.vector.bn_stats(
                    out=stats[:, ch, :], in_=xc[t][:, ch * 512:(ch + 1) * 512]
                )
            mv = work.tile([P, 2], f32, tag="mv")
            nc.vector.bn_aggr(out=mv, in_=stats)
            # mv col1 := var + mean^2  (= E[x^2] per channel)
            msq = work.tile([P, 1], f32, tag="msq")
            nc.vector.tensor_mul(msq, mv[:, 0:1], mv[:, 0:1])
            nc.vector.tensor_add(mv[:, 1:2], mv[:, 1:2], msq)
            # per-group (mean, E[x^2]) via block-ones (1/8) matmul
            gst_ps = psB.tile([16, 2], f32, tag="dn")
            nc.tensor.matmul(gst_ps, lhsT=blk8_sb, rhs=mv, start=True, stop=True)
            gst = work.tile([16, 2], f32, tag="gst")
            nc.vector.tensor_copy(gst, gst_ps)
            mmg = work.tile([16, 1], f32, tag="mmg")
            nc.vector.tensor_mul(mmg, gst[:, 0:1], gst[:, 0:1])
            varg = work.tile([16, 1], f32, tag="varg")
            nc.vector.tensor_sub(varg, gst[:, 1:2], mmg)
            sdg = work.tile([16, 1], f32, tag="sdg")
            nc.scalar.activation(
                out=sdg, in_=varg, func=AF.Sqrt, bias=eps_sb, scale=1.0
            )
            ms = work.tile([16, 2], f32, tag="ms")
            nc.vector.tensor_copy(ms[:, 0:1], gst[:, 0:1])
            nc.vector.reciprocal(ms[:, 1:2], sdg)
            # broadcast group (mean, rstd) to the 8 channels of each group
            cb_ps = psB.tile([P, 2], f32, tag="oacc")
            nc.tensor.matmul(cb_ps, lhsT=blk8T_sb, rhs=ms, start=True, stop=True)
            al = persist.tile([P, 1], f32, tag=f"alpha{t}")
            nc.vector.tensor_mul(al, cb_ps[:, 1:2], gnw_sb[t])
            tmpb = work.tile([P, 1], f32, tag="tmpb")
            nc.vector.tensor_mul(tmpb, cb_ps[:, 0:1], al)
            be = persist.tile([P, 1], f32, tag=f"beta{t}")
            nc.vector.tensor_sub(be, gnb_sb[t], tmpb)
            # xn = x * alpha + beta  (in place)
            nc.vector.tensor_scalar(
                out=xc[t], in0=xc[t], scalar1=al, scalar2=be,
                op0=ALU.mult, op1=ALU.add,
            )

        # ---------------- QKV projections ----------------
        qT = persist.tile([P, NQ], f32, tag="qT")
        kT = persist.tile([P, N], f32, tag="kT")
        vS = persist.tile([P, N], f32, tag="vS")   # vS[p, i*128+o] = v[i*128+p, o]
        for jq in range(NQ // 512):
            ps = psA.tile([P, 2, JW], f32, tag="sim")
            for t in range(2):
                nc.tensor.matmul(
                    ps[:, 0, :], lhsT=wqkv_sb[t][:, 0:P],
                    rhs=xc[t][:, jq * 512:(jq + 1) * 512],
                    start=(t == 0), stop=(t == 1),
                )
            nc.vector.tensor_copy(qT[:, jq * 512:(jq + 1) * 512], ps[:, 0, :])
        for jk in range(N // 512):
            ps = psA.tile([P, 2, JW], f32, tag="sim")
            for t in range(2):
                nc.tensor.matmul(
                    ps[:, 0, :], lhsT=wqkv_sb[t][:, P:2 * P],
                    rhs=xc[t][:, jk * 512:(jk + 1) * 512],
                    start=(t == 0), stop=(t == 1),
                )
            nc.vector.tensor_copy(kT[:, jk * 512:(jk + 1) * 512], ps[:, 0, :])
        for i in range(NI):
            ps = psB.tile([P, P], f32, tag="oacc")
            for t in range(2):
                nc.tensor.matmul(
                    ps, lhsT=xc[t][:, i * P:(i + 1) * P],
                    rhs=wqkv_sb[t][:, 2 * P:3 * P],
                    start=(t == 0), stop=(t == 1),
                )
            nc.vector.tensor_copy(vS[:, i * P:(i + 1) * P], ps)

        # ---------------- attention ----------------
        for j in range(NJ):
            oacc = psB.tile([P, JW], f32, tag="oacc")
            dn = psB.tile([P, JW], f32, tag="dn")
            for i in range(NI):
                ats = []
                for pr in range(2):           # head pairs -> 2-bank sim tiles
                    sim = psA.tile([P, 2, JW], f32, tag="sim")
                    for hh in range(2):
                        h = pr * 2 + hh
                        nc.tensor.matmul(
                            sim[:, hh, :],
                            lhsT=kT[32 * h:32 * h + 32, i * P:(i + 1) * P],
                            rhs=qT[32 * h:32 * h + 32, j * JW:(j + 1) * JW],
                            start=True, stop=True,
                            tile_position=(32 * h, 0),
                        )
                    at = attnp.tile([P, 2, JW], f32, tag="attn")
                    nc.scalar.activation(out=at, in_=sim, func=AF.Exp, scale=SCALE)
                    ats.append(at)
                for pr in range(2):
                    for hh in range(2):
                        h = pr * 2 + hh
                        nc.tensor.matmul(
                            oacc[32 * h:32 * h + 32, :],
                            lhsT=vS[:, i * P + 32 * h:i * P + 32 * h + 32],
                            rhs=ats[pr][:, hh, :],
                            start=(i == 0), stop=(i == NI - 1),
                            tile_position=(0, 32 * h),
                            skip_group_check=True,
                        )
                for pr in range(2):
                    for hh in range(2):
                        h = pr * 2 + hh
                        nc.tensor.matmul(
                            dn[32 * h:32 * h + 1, :],
                            lhsT=ones_sb,
                            rhs=ats[pr][:, hh, :],
                            start=(i == 0), stop=(i == NI - 1),
                            tile_position=(0, 32 * h),
                            skip_group_check=True,
                        )

            # ---- per-j epilogue: divide by denominator, project, bias ----
            # stage denom rows (psum partitions 0/32/64/96) into dnc (pre-set
            # to 1.0 so unwritten rows stay finite), select+broadcast each
            # head's row to its 32 hidden partitions, then one reciprocal.
            for h in range(HEAD):
                nc.vector.tensor_copy(
                    dnc[32 * h:32 * h + 1, :], dn[32 * h:32 * h + 1, :]
                )
            dbc_ps = psA.tile([P, 2, JW], f32, tag="sim")
            nc.tensor.matmul(
                dbc_ps[:, 0, :], lhsT=e4_sb, rhs=dnc, start=True, stop=True
            )
            rcb = work.tile([P, JW], f32, tag="rcb")
            nc.vector.reciprocal(rcb, dbc_ps[:, 0, :])
            ao = work.tile([P, JW], f32, tag="ao")
            nc.vector.tensor_mul(ao, oacc, rcb)
            for t in range(2):
                yps = psB.tile([P, JW], f32, tag="dn")
                nc.tensor.matmul(
                    yps, lhsT=wout_sb[:, t * P:(t + 1) * P], rhs=ao,
                    start=True, stop=True,
                )
                ysb = work.tile([P, JW], f32, tag="ysb")
                nc.vector.tensor_scalar_add(ysb, yps, bout_sb[t])
                nc.sync.dma_start(
                    out=y_out[t * P:(t + 1) * P, j * JW:(j + 1) * JW], in_=ysb
                )

    nc.finalize()
    _cache["nc"] = nc
    return nc


def _prep_in_maps(x, gn_weight, gn_bias, w_qkv, w_out, b_out):
    f = np.float32
    x = np.asarray(x, dtype=f)
    wqkvT = np.ascontiguousarray(np.asarray(w_qkv, dtype=f).T)
    woutT = np.ascontiguousarray(np.asarray(w_out, dtype=f).T)
    gnw = np.ascontiguousarray(np.asarray(gn_weight, dtype=f).reshape(DIM, 1))
    gnb = np.ascontiguousarray(np.asarray(gn_bias, dtype=f).reshape(DIM, 1))
    bo = np.ascontiguousarray(np.asarray(b_out, dtype=f).reshape(DIM, 1))
    ar = np.arange(P)
    blk8 = np.zeros((P, 16), f)
    blk8[ar, ar // 8] = 0.125
    blk8T = np.zeros((16, P), f)
    blk8T[ar // 8, ar] = 1.0
    # selector/broadcast: out[q] = in[32*(q//32)] — picks each head's denom
    # row (at partition 32h) and fans it out to that head's 32 partitions
    e4 = np.zeros((P, P), f)
    e4[32 * (ar // 32), ar] = 1.0
    onesP = np.ones((P, 1), f)
    shared = dict(wqkvT=wqkvT, woutT=woutT, gnw=gnw, gnb=gnb, bout=bo,
                  blk8=blk8, blk8T=blk8T, e4=e4, onesP=onesP)
    in_maps = []
    for core in range(NCORES):
        b, half = divmod(core, 2)
        xb = x[b].reshape(DIM, N)
        if half == 0:
            xp = np.ascontiguousarray(xb)
        else:
            xp = np.ascontiguousarray(
                np.concatenate([xb[:, NQ:], xb[:, :NQ]], axis=1)
            )
        in_maps.append(dict(x=xp, **shared))
    return in_maps


def _get_executor():
    """Build the sharded jitted executor once (compiles the NEFF once).

    Returns (exec_fn, meta): exec_fn takes a list of 8 per-core input dicts
    and returns the list of 8 per-core output dicts.  Mirrors
    concourse.bass2jax.run_bass_via_pjrt's multi-core path but caches the
    jax.jit so repeated calls don't recompile.
    """
    if "exec" in _cache:
        return _cache["exec"]
    import jax
    import concourse.mybir as mybir
    from jax.sharding import Mesh, PartitionSpec
    from jax.experimental.shard_map import shard_map
    from concourse import bass2jax

    bass2jax.install_neuronx_cc_hook()
    nc = _get_nc()

    partition_name = (
        nc.partition_id_tensor.name if nc.partition_id_tensor else None
    )
    in_names, out_names, out_avals, zero_outs = [], [], [], []
    for alloc in nc.m.functions[0].allocations:
        if not isinstance(alloc, mybir.MemoryLocationSet):
            continue
        name = alloc.memorylocations[0].name
        if alloc.kind == "ExternalInput":
            if name != partition_name:
                in_names.append(name)
        elif alloc.kind == "ExternalOutput":
            shape = tuple(alloc.tensor_shape)
            dtype = mybir.dt.np(alloc.dtype)
            out_names.append(name)
            out_avals.append(jax.core.ShapedArray(shape, dtype))
            zero_outs.append(np.zeros(shape, dtype))
    n_params = len(in_names)
    n_outs = len(out_names)
    all_names = in_names + out_names
    if partition_name is not None:
        all_names = all_names + [partition_name]
    donate = tuple(range(n_params, n_params + n_outs))

    def _body(*args):
        operands = list(args)
        if partition_name is not None:
            operands.append(bass2jax.partition_id_tensor())
        outs = bass2jax._bass_exec_p.bind(
            *operands,
            out_avals=tuple(out_avals),
            in_names=tuple(all_names),
            out_names=tuple(out_names),
            lowering_input_output_aliases=(),
            sim_require_finite=True,
            sim_require_nnan=True,
            nc=nc,
        )
        return tuple(outs)

    devices = jax.devices()[:NCORES]
    mesh = Mesh(np.asarray(devices), ("core",))
    sharded = jax.jit(
        shard_map(
            _body, mesh=mesh,
            in_specs=(PartitionSpec("core"),) * (n_params + n_outs),
            out_specs=(PartitionSpec("core"),) * n_outs,
            check_rep=False,
        ),
        donate_argnums=donate, keep_unused=True,
    )

    def exec_fn(in_maps, device_inputs=None):
        if device_inputs is None:
            device_inputs = [
                np.concatenate([np.asarray(m[name]) for m in in_maps], axis=0)
                for name in in_names
            ]
        concat_zeros = [
            np.zeros((NCORES * z.shape[0], *z.shape[1:]), z.dtype)
            for z in zero_outs
        ]
        out_arrs = sharded(*device_inputs, *concat_zeros)
        out_arrs = [np.asarray(a) for a in out_arrs]
        return [
            {
                name: out_arrs[i].reshape(NCORES, *out_avals[i].shape)[c]
                for i, name in enumerate(out_names)
            }
            for c in range(NCORES)
        ]

    meta = dict(in_names=in_names, out_names=out_names, mesh=mesh,
                sharded=sharded, zero_outs=zero_outs)
    _cache["exec"] = (exec_fn, meta)
    return _cache["exec"]


def _assemble(results):
    y = np.empty((4, DIM, N), np.float32)
    for core in range(NCORES):
        b, half = divmod(core, 2)
        y[b][:, half * NQ:(half + 1) * NQ] = results[core]["y"]
    return y.reshape(4, DIM, 64, 64)


def _run(inputs, **kw):
    exec_fn, _ = _get_executor()
    in_maps = _prep_in_maps(**inputs)
    results = exec_fn(in_maps)
    return _assemble(results), results


def kernel(x, gn_weight, gn_bias, w_qkv, w_out, b_out):
    out, _ = _run(dict(x=x, gn_weight=gn_weight, gn_bias=gn_bias,
                       w_qkv=w_qkv, w_out=w_out, b_out=b_out))
    return out
